# revision 1
# baseline (speedup 1.0000x reference)
"""Autoregressive LSTM decompressor on 8 Trainium2 NeuronCores.

Math (from the reference): output h of each step feeds back as the next
step's input, so for t>=1 the two matmuls collapse into one with
W_sum = W_ih + W_hh:
    gates_0 = x @ W_ih.T + b          (h0 = c0 = 0)
    gates_t = h_{t-1} @ W_sum.T + b   (t >= 1)
    i,f,g,o = split(gates); c = sig(f)*c + sig(i)*tanh(g); h = sig(o)*tanh(c)
    y = stack(h_0..h_{L-1}) @ W_out.T + b_out

Sharding: tensor-parallel over the 4D gate dim. Core c owns hidden units
[256c, 256c+256) and the matching 1024 rows of W_ih/W_sum. Each step every
core does a [2048] x [2048,1024] matvec from SBUF-resident bf16 weights
(weights as the *moving* PE operand, h chunks as the tiny stationary
operand), the gate nonlinearities, then an 8-core AllGather of its 256
bf16 h values so every core has the full h for the next step. The final
W_out projection is computed redundantly on every core (it is tiny).

Bias is folded into the matvec as a 17th contraction chunk against a
constant e0 column. b_out is added on the host.
"""

import numpy as np
import ml_dtypes

D = 2048          # hidden/input width
DOUT = 1024       # output width
L = 256           # seq_len
NCORES = 8
KC = 16           # contraction chunks of 128
UPC = D // NCORES  # units per core = 256

_BF16 = ml_dtypes.bfloat16

# contraction slot (p, k) -> global hidden unit id
# chosen so the AllGather output (rank-major concat of 256-unit slices,
# slice position = local unit id) is already partition-major for a plain
# [128, 16] DMA into SBUF: unit u = 256*(p//16) + (p%16)*16 + k
_P = np.arange(128)
_K = np.arange(KC)
_UMAP = (_P[:, None] // 16) * 256 + (_P[:, None] % 16) * 16 + _K[None, :]  # [128,16]


def _prep_core_inputs(x, W_ih, W_sum, b, W_out):
    """Build the per-core host-side input dict list."""
    in_maps = []
    for c in range(NCORES):
        sl = np.arange(UPC) + UPC * c
        # gate free-dim order [i | f | o | g] (sigmoid group contiguous);
        # W row blocks in reference order are i,f,g,o
        rows = np.concatenate([sl, sl + 2048, sl + 6144, sl + 4096])
        wrec = np.zeros((128, KC + 1, 1024), np.float32)
        wrec[:, :KC, :] = np.transpose(W_sum[rows][:, _UMAP], (1, 2, 0))
        wrec[0, KC, :] = b[rows]
        wx = np.zeros((128, KC + 1, 1024), np.float32)
        wx[:, :KC, :] = np.transpose(W_ih[rows][:, _UMAP], (1, 2, 0))
        wx[0, KC, :] = b[rows]
        wout = np.transpose(W_out[:, _UMAP], (1, 2, 0))  # [128,16,1024]
        xp = np.zeros((128, KC + 1), np.float32)
        xp[:, :KC] = x[0][_UMAP]
        xp[0, KC] = 1.0
        in_maps.append({
            "wrec": wrec.astype(_BF16),
            "wx": wx.astype(_BF16),
            "wout": wout.astype(_BF16),
            "xp": xp.astype(_BF16),
        })
    return in_maps


def _build_program(reps=1, nsteps=L):
    from concourse import bacc, tile, mybir

    dt = mybir.dt
    nc = bacc.Bacc("TRN2", target_bir_lowering=False, debug=False,
                   num_devices=NCORES)

    wrec_d = nc.dram_tensor("wrec", [128, KC + 1, 1024], dt.bfloat16,
                            kind="ExternalInput")
    wx_d = nc.dram_tensor("wx", [128, KC + 1, 1024], dt.bfloat16,
                          kind="ExternalInput")
    wout_d = nc.dram_tensor("wout", [128, KC, 1024], dt.bfloat16,
                            kind="ExternalInput")
    xp_d = nc.dram_tensor("xp", [128, KC + 1], dt.bfloat16,
                          kind="ExternalInput")
    y_d = nc.dram_tensor("y", [nsteps, DOUT], dt.float32,
                         kind="ExternalOutput")

    Sig = mybir.ActivationFunctionType.Sigmoid
    Tanh = mybir.ActivationFunctionType.Tanh
    rg = [list(range(NCORES))]

    with tile.TileContext(nc) as tc:
        with (
            tc.tile_pool(name="wpool", bufs=1) as wpool,
            tc.tile_pool(name="state", bufs=1) as state,
            tc.tile_pool(name="work", bufs=3) as work,
            tc.tile_pool(name="psum", bufs=2, space="PSUM") as psum,
            tc.tile_pool(name="dram", bufs=3, space="DRAM") as dram,
        ):
            wrec = wpool.tile([128, KC + 1, 1024], dt.bfloat16)
            wx = wpool.tile([128, KC + 1, 1024], dt.bfloat16)
            wout = wpool.tile([128, KC, 1024], dt.bfloat16)
            xp = wpool.tile([128, KC + 1], dt.bfloat16)
            hist = state.tile([128, nsteps, KC], dt.bfloat16)
            # tc_cat = [tanh(g) | c]: lets one 512-wide DVE mul compute both
            # i*tanh(g) and f*c; c lives persistently in the second half
            tc_cat = state.tile([1, 2 * UPC], dt.float32)

            nc.sync.dma_start(wrec[:], wrec_d[:])
            nc.sync.dma_start(wx[:], wx_d[:])
            nc.sync.dma_start(wout[:], wout_d[:])
            nc.sync.dma_start(xp[:], xp_d[:])
            nc.vector.memset(tc_cat[:], 0.0)

            for s in range(nsteps * reps):
                t = s % nsteps
                g0 = psum.tile([1, 512], dt.float32, tag="g0")
                g1 = psum.tile([1, 512], dt.float32, tag="g1")
                W = wx if s == 0 else wrec
                # bias/ones chunk first: it has no dependency on the
                # exchanged h, so the PE can issue it while the AllGather
                # is still in flight (keeps the PE warm, off critical path)
                for k in [KC] + list(range(KC)):
                    if s == 0:
                        lhsT = xp[:, k:k + 1]
                    elif k < KC:
                        lhsT = hist[:, (s - 1) % nsteps, k:k + 1]
                    else:
                        lhsT = xp[:, KC:KC + 1]
                    nc.tensor.matmul(g0[:], lhsT, W[:, k, 0:512],
                                     start=(k == KC), stop=(k == KC - 1))
                    nc.tensor.matmul(g1[:], lhsT, W[:, k, 512:1024],
                                     start=(k == KC), stop=(k == KC - 1))

                # gates free-dim: g0 = [i|f], g1 = [o|g̃]
                s_if = work.tile([1, 512], dt.float32, tag="sif")
                s_o = work.tile([1, UPC], dt.float32, tag="so")
                nc.scalar.activation(s_if[:], g0[:], Sig)
                nc.scalar.activation(tc_cat[:, 0:UPC], g1[:, UPC:2 * UPC],
                                     Tanh)
                nc.scalar.activation(s_o[:], g1[:, 0:UPC], Sig)

                m12 = work.tile([1, 512], dt.float32, tag="m12")
                nc.vector.tensor_mul(m12[:], s_if[:], tc_cat[:])
                nc.vector.tensor_add(tc_cat[:, UPC:2 * UPC],
                                     m12[:, 0:UPC], m12[:, UPC:2 * UPC])
                t_c = work.tile([1, UPC], dt.float32, tag="tc")
                nc.scalar.activation(t_c[:], tc_cat[:, UPC:2 * UPC], Tanh)
                h_bf = work.tile([1, UPC], dt.bfloat16, tag="hbf")
                nc.vector.tensor_mul(h_bf[:], s_o[:], t_c[:])

                inb = dram.tile([1, UPC], dt.bfloat16, tag="inb")
                outb = dram.tile([128, KC], dt.bfloat16, tag="outb")
                nc.sync.dma_start(inb[:], h_bf[:])
                nc.gpsimd.collective_compute(
                    "AllGather", mybir.AluOpType.bypass,
                    ins=[inb.opt()], outs=[outb.opt()], replica_groups=rg)
                nc.sync.dma_start(hist[:, t, :], outb[:])

            # output projection: y = H @ W_out.T (redundant on every core)
            schunk = min(nsteps, 128)
            for sh in range(nsteps // schunk):
                for nh in range(2):
                    yp = psum.tile([schunk, 512], dt.float32, tag="yp")
                    for k in range(KC):
                        nc.tensor.matmul(
                            yp[:], hist[:, sh * schunk:(sh + 1) * schunk, k],
                            wout[:, k, nh * 512:(nh + 1) * 512],
                            start=(k == 0), stop=(k == KC - 1))
                    y_sb = work.tile([schunk, 512], dt.float32, tag="ysb")
                    nc.vector.tensor_copy(y_sb[:], yp[:])
                    nc.sync.dma_start(
                        y_d[sh * schunk:(sh + 1) * schunk,
                            nh * 512:(nh + 1) * 512],
                        y_sb[:])

    nc.compile()
    return nc


def kernel(x, W_ih, W_hh, b_ih, b_hh, W_out, b_out, seq_len, _trace=False):
    from concourse.bass_utils import run_bass_kernel_spmd

    assert int(seq_len) == L
    x = np.asarray(x, np.float32)
    W_ih = np.asarray(W_ih, np.float32)
    W_hh = np.asarray(W_hh, np.float32)
    b = np.asarray(b_ih, np.float32) + np.asarray(b_hh, np.float32)
    W_out = np.asarray(W_out, np.float32)
    b_out = np.asarray(b_out, np.float32)

    in_maps = _prep_core_inputs(x, W_ih, W_ih + W_hh, b, W_out)
    nc = _build_program()
    res = run_bass_kernel_spmd(nc, in_maps, list(range(NCORES)),
                               trace=_trace)
    y = np.asarray(res.results[0]["y"], np.float32) + b_out
    out = y[None]  # [1, L, DOUT]
    if _trace:
        return out, res
    return out



# revision 2
# speedup vs baseline: 1.0598x; 1.0598x over previous
"""Autoregressive LSTM decompressor on 8 Trainium2 NeuronCores (v3).

Same sharding/exchange skeleton as the baseline (tensor-parallel over the
gate dim, per-step AllGather of each core's 256 h values, SBUF-resident
bf16 weights, bias folded as a 17th contraction chunk), plus:

- gate-type column groups: the four gate blocks i/f/o/g~ are computed by
  four PE column-groups (tile_position), landing on psum partitions
  0/32/64/96. One sigmoid activation over partitions 0..64 covers i,f,o
  in a single ACT pass; g~ is emitted first so tanh(g) overlaps the
  remaining matmuls.
- mate tile holds [tanh(g) @p0, c @p32] so a single 33-partition DVE mul
  produces i*tanh(g) and f*c together.

y = H @ W_out.T runs once at the end (redundant per core); b_out is
added on the host.
"""

import numpy as np
import ml_dtypes

D = 2048          # hidden/input width
DOUT = 1024       # output width
L = 256           # seq_len
NCORES = 8
KC = 16           # contraction chunks of 128
UPC = D // NCORES  # units per core = 256

_BF16 = ml_dtypes.bfloat16

# contraction slot (p, k) -> global hidden unit id
# chosen so the AllGather output (rank-major concat of 256-unit slices,
# slice position = local unit id) is already partition-major for a plain
# [128, 16] DMA into SBUF: unit u = 256*(p//16) + (p%16)*16 + k
_P = np.arange(128)
_K = np.arange(KC)
_UMAP = (_P[:, None] // 16) * 256 + (_P[:, None] % 16) * 16 + _K[None, :]  # [128,16]


def _prep_core_inputs(x, W_ih, W_sum, b, W_out):
    """Build the per-core host-side input dict list."""
    in_maps = []
    for c in range(NCORES):
        sl = np.arange(UPC) + UPC * c
        # gate free-dim order [i | f | o | g] (psum col-groups 0..3);
        # W row blocks in reference order are i,f,g,o
        rows = np.concatenate([sl, sl + 2048, sl + 6144, sl + 4096])
        wrec = np.zeros((128, KC + 1, 1024), np.float32)
        wrec[:, :KC, :] = np.transpose(W_sum[rows][:, _UMAP], (1, 2, 0))
        wrec[0, KC, :] = b[rows]
        wx = np.zeros((128, KC + 1, 1024), np.float32)
        wx[:, :KC, :] = np.transpose(W_ih[rows][:, _UMAP], (1, 2, 0))
        wx[0, KC, :] = b[rows]
        wout = np.transpose(W_out[:, _UMAP], (1, 2, 0))  # [128,16,1024]
        xp = np.zeros((128, KC + 1), np.float32)
        xp[:, :KC] = x[0][_UMAP]
        xp[0, KC] = 1.0
        in_maps.append({
            "wrec": wrec.astype(_BF16),
            "wx": wx.astype(_BF16),
            "wout": wout.astype(_BF16),
            "xp": xp.astype(_BF16),
        })
    return in_maps


def _build_program(nsteps=L):
    from concourse import bacc, tile, mybir

    dt = mybir.dt
    nc = bacc.Bacc("TRN2", target_bir_lowering=False, debug=False,
                   num_devices=NCORES)

    wrec_d = nc.dram_tensor("wrec", [128, KC + 1, 1024], dt.bfloat16,
                            kind="ExternalInput")
    wx_d = nc.dram_tensor("wx", [128, KC + 1, 1024], dt.bfloat16,
                          kind="ExternalInput")
    wout_d = nc.dram_tensor("wout", [128, KC, 1024], dt.bfloat16,
                            kind="ExternalInput")
    xp_d = nc.dram_tensor("xp", [128, KC + 1], dt.bfloat16,
                          kind="ExternalInput")
    y_d = nc.dram_tensor("y", [nsteps, DOUT], dt.float32,
                         kind="ExternalOutput")

    Sig = mybir.ActivationFunctionType.Sigmoid
    Tanh = mybir.ActivationFunctionType.Tanh
    rg = [list(range(NCORES))]

    with tile.TileContext(nc) as tc:
        with (
            tc.tile_pool(name="wpool", bufs=1) as wpool,
            tc.tile_pool(name="state", bufs=1) as state,
            tc.tile_pool(name="work", bufs=3) as work,
            tc.tile_pool(name="psum", bufs=2, space="PSUM") as psum,
            tc.tile_pool(name="dram", bufs=3, space="DRAM") as dram,
        ):
            wrec = wpool.tile([128, KC + 1, 1024], dt.bfloat16)
            wx = wpool.tile([128, KC + 1, 1024], dt.bfloat16)
            wout = wpool.tile([128, KC, 1024], dt.bfloat16)
            xp = wpool.tile([128, KC + 1], dt.bfloat16)
            hist = state.tile([128, nsteps, KC], dt.bfloat16)
            # mate[0] = tanh(g), mate[32] = c: one 33-partition DVE mul
            # computes i*tanh(g) and f*c together
            mate = state.tile([33, UPC], dt.float32)
            # persistent double-buffered gates psum (one memset each, so
            # the [0:65]-wide ACT read below never touches uninitialized
            # PSUM rows between the 4 col-group output partitions)
            gps = [psum.tile([128, 256], dt.float32, tag=f"gpersist{i}",
                             bufs=1, name=f"gpersist{i}") for i in range(2)]
            # separate psum tiles for the g~ group so tanh(g) only
            # depends on the 17 g~ matmuls (fires while i/f/o still run)
            gts = [psum.tile([128, 256], dt.float32, tag=f"gtp{i}",
                             bufs=1, name=f"gtp{i}") for i in range(2)]
            # filler target: keeps the PE pstate warm across the
            # AllGather window (never read)
            fpp = psum.tile([128, 512], dt.float32, tag="fpp", bufs=1,
                            name="fpp")
            flh = state.tile([128, 1], dt.bfloat16)

            nc.sync.dma_start(wrec[:], wrec_d[:])
            nc.sync.dma_start(wx[:], wx_d[:])
            nc.sync.dma_start(wout[:], wout_d[:])
            nc.sync.dma_start(xp[:], xp_d[:])
            nc.vector.memset(mate[:], 0.0)
            nc.vector.memset(flh[:], 0.0)
            for g in gps:
                nc.vector.memset(g[:], 0.0)

            NFILL = 105
            for s in range(nsteps):
                W = wx if s == 0 else wrec
                gp = gps[s % 2]
                gt = gts[s % 2]
                # group-major, g~ first so tanh(g) overlaps i/f/o MMs;
                # bias/ones chunk first within each group: it has no
                # dependency on the exchanged h, so the PE can issue it
                # while the AllGather is still in flight
                for gi in [3, 0, 1, 2]:
                    dst = gt if gi == 3 else gp
                    for k in [KC] + list(range(KC)):
                        if s == 0:
                            lhsT = xp[:, k:k + 1]
                        elif k < KC:
                            lhsT = hist[:, s - 1, k:k + 1]
                        else:
                            lhsT = xp[:, KC:KC + 1]
                        nc.tensor.matmul(
                            dst[32 * gi:32 * gi + 1, :], lhsT,
                            W[:, k, 256 * gi:256 * gi + 256],
                            start=(k == KC), stop=(k == KC - 1),
                            tile_position=(0, 32 * gi))

                # ---- LSTM cell elementwise ----
                # walrus requires equal base partitions when both DVE
                # inputs are in SBUF, so each two-input op below reads
                # operands at one base and redirects its output freely.
                s_ifo = work.tile([65, UPC], dt.float32, tag="sifo")
                nc.scalar.activation(mate[0:1, :], gt[96:97, :], Tanh)
                nc.scalar.activation(s_ifo[:], gp[0:65, :], Sig)
                m_ig = work.tile([1, UPC], dt.float32, tag="mig")
                m_fc = work.tile([1, UPC], dt.float32, tag="mfc")
                nc.vector.tensor_mul(m_ig[:], s_ifo[0:1, :], mate[0:1, :])
                nc.vector.tensor_mul(m_fc[:], s_ifo[32:33, :],
                                     mate[32:33, :])
                nc.vector.tensor_add(mate[32:33, :], m_ig[:], m_fc[:])
                t_c = work.tile([65, UPC], dt.float32, tag="tc")
                nc.scalar.activation(t_c[64:65, :], mate[32:33, :], Tanh)
                h_bf = work.tile([1, UPC], dt.bfloat16, tag="hbf")
                nc.vector.tensor_mul(h_bf[:], s_ifo[64:65, :],
                                     t_c[64:65, :])

                inb = dram.tile([1, UPC], dt.bfloat16, tag="inb")
                outb = dram.tile([128, KC], dt.bfloat16, tag="outb")
                nc.sync.dma_start(inb[:], h_bf[:])
                nc.gpsimd.collective_compute(
                    "AllGather", mybir.AluOpType.bypass,
                    ins=[inb.opt()], outs=[outb.opt()], replica_groups=rg)
                nc.sync.dma_start(hist[:, s, :], outb[:])
                if s < nsteps - 1:
                    # dummy matmuls bridge the PE through the AllGather
                    # window so the real matmul burst starts at full
                    # p-state (cost model warm-up)
                    for _ in range(NFILL):
                        nc.tensor.matmul(fpp[0:1, :], flh[:, 0:1],
                                         wrec[:, KC, 0:512],
                                         start=True, stop=True)

            # output projection: y = H @ W_out.T (redundant on every core)
            for sh in range(nsteps // 128):
                for nh in range(2):
                    yp = psum.tile([128, 512], dt.float32, tag="yp")
                    for k in range(KC):
                        nc.tensor.matmul(
                            yp[:], hist[:, sh * 128:(sh + 1) * 128, k],
                            wout[:, k, nh * 512:(nh + 1) * 512],
                            start=(k == 0), stop=(k == KC - 1))
                    y_sb = work.tile([128, 512], dt.float32, tag="ysb")
                    nc.vector.tensor_copy(y_sb[:], yp[:])
                    nc.sync.dma_start(
                        y_d[sh * 128:(sh + 1) * 128,
                            nh * 512:(nh + 1) * 512],
                        y_sb[:])

    nc.compile()
    return nc


def kernel(x, W_ih, W_hh, b_ih, b_hh, W_out, b_out, seq_len, _trace=False):
    from concourse.bass_utils import run_bass_kernel_spmd

    assert int(seq_len) == L
    x = np.asarray(x, np.float32)
    W_ih = np.asarray(W_ih, np.float32)
    W_hh = np.asarray(W_hh, np.float32)
    b = np.asarray(b_ih, np.float32) + np.asarray(b_hh, np.float32)
    W_out = np.asarray(W_out, np.float32)
    b_out = np.asarray(b_out, np.float32)

    in_maps = _prep_core_inputs(x, W_ih, W_ih + W_hh, b, W_out)
    nc = _build_program()
    res = run_bass_kernel_spmd(nc, in_maps, list(range(NCORES)),
                               trace=_trace)
    y = np.asarray(res.results[0]["y"], np.float32) + b_out
    out = y[None]  # [1, L, DOUT]
    if _trace:
        return out, res
    return out


# revision 4
# speedup vs baseline: 1.0995x; 1.0375x over previous
"""Autoregressive LSTM decompressor on 8 Trainium2 NeuronCores (v3).

Same sharding/exchange skeleton as the baseline (tensor-parallel over the
gate dim, per-step AllGather of each core's 256 h values, SBUF-resident
bf16 weights, bias folded as a 17th contraction chunk), plus:

- gate-type column groups: the four gate blocks i/f/o/g~ are computed by
  four PE column-groups (tile_position) on psum partitions 0/32/64/96,
  split across two psum tiles so the g~/f/i activations and the whole
  c update run while the o-group matmuls still stream; only sigma(o)
  and the final h multiply trail the burst.
- dummy matmuls bridge the PE through each AllGather window so the real
  burst always runs at the warm p-state.

y = H @ W_out.T runs once at the end (redundant per core); b_out is
added on the host.
"""

import numpy as np
import ml_dtypes

D = 2048          # hidden/input width
DOUT = 1024       # output width
L = 256           # seq_len
NCORES = 8
KC = 16           # contraction chunks of 128
UPC = D // NCORES  # units per core = 256

_BF16 = ml_dtypes.bfloat16

# contraction slot (p, k) -> global hidden unit id
# chosen so the AllGather output (rank-major concat of 256-unit slices,
# slice position = local unit id) is already partition-major for a plain
# [128, 16] DMA into SBUF: unit u = 256*(p//16) + (p%16)*16 + k
_P = np.arange(128)
_K = np.arange(KC)
_UMAP = (_P[:, None] // 16) * 256 + (_P[:, None] % 16) * 16 + _K[None, :]  # [128,16]


def _prep_core_inputs(x, W_ih, W_sum, b, W_out):
    """Build the per-core host-side input dict list."""
    in_maps = []
    for c in range(NCORES):
        sl = np.arange(UPC) + UPC * c
        # gate free-dim order [i | f | o | g] (psum col-groups 0..3);
        # W row blocks in reference order are i,f,g,o
        rows = np.concatenate([sl, sl + 2048, sl + 6144, sl + 4096])
        wrec = np.zeros((128, KC + 1, 1024), np.float32)
        wrec[:, :KC, :] = np.transpose(W_sum[rows][:, _UMAP], (1, 2, 0))
        wrec[0, KC, :] = b[rows]
        wx = np.zeros((128, KC + 1, 1024), np.float32)
        wx[:, :KC, :] = np.transpose(W_ih[rows][:, _UMAP], (1, 2, 0))
        wx[0, KC, :] = b[rows]
        wout = np.transpose(W_out[:, _UMAP], (1, 2, 0))  # [128,16,1024]
        xp = np.zeros((128, KC + 1), np.float32)
        xp[:, :KC] = x[0][_UMAP]
        xp[0, KC] = 1.0
        in_maps.append({
            "wrec": wrec.astype(_BF16),
            "wx": wx.astype(_BF16),
            "wout": wout.astype(_BF16),
            "xp": xp.astype(_BF16),
        })
    return in_maps


def _build_program(nsteps=L):
    from concourse import bacc, tile, mybir

    dt = mybir.dt
    nc = bacc.Bacc("TRN2", target_bir_lowering=False, debug=False,
                   num_devices=NCORES)

    wrec_d = nc.dram_tensor("wrec", [128, KC + 1, 1024], dt.bfloat16,
                            kind="ExternalInput")
    wx_d = nc.dram_tensor("wx", [128, KC + 1, 1024], dt.bfloat16,
                          kind="ExternalInput")
    wout_d = nc.dram_tensor("wout", [128, KC, 1024], dt.bfloat16,
                            kind="ExternalInput")
    xp_d = nc.dram_tensor("xp", [128, KC + 1], dt.bfloat16,
                          kind="ExternalInput")
    y_d = nc.dram_tensor("y", [nsteps, DOUT], dt.float32,
                         kind="ExternalOutput")

    Sig = mybir.ActivationFunctionType.Sigmoid
    Tanh = mybir.ActivationFunctionType.Tanh
    rg = [list(range(NCORES))]

    with tile.TileContext(nc) as tc:
        with (
            tc.tile_pool(name="wpool", bufs=1) as wpool,
            tc.tile_pool(name="state", bufs=1) as state,
            tc.tile_pool(name="work", bufs=3) as work,
            tc.tile_pool(name="psum", bufs=2, space="PSUM") as psum,
            tc.tile_pool(name="dram", bufs=3, space="DRAM") as dram,
        ):
            wrec = wpool.tile([128, KC + 1, 1024], dt.bfloat16)
            wx = wpool.tile([128, KC + 1, 1024], dt.bfloat16)
            wout = wpool.tile([128, KC, 1024], dt.bfloat16)
            xp = wpool.tile([128, KC + 1], dt.bfloat16)
            hist = state.tile([128, nsteps, KC], dt.bfloat16)
            # c state, base partition 0 (all two-input DVE ops below keep
            # both inputs at base 0 — walrus requires equal SBUF bases)
            cst = state.tile([1, UPC], dt.float32)
            # one psum tile pair per gate group: each gate's activation
            # only depends on that group's 17 matmuls, so with group
            # order g~,f,i,o the whole cell state update runs during the
            # i/o matmuls and only sigma(o)*tanh(c) trails the burst
            _gA = [psum.tile([128, 256], dt.float32, tag=f"gA{i}",
                             bufs=1, name=f"gA{i}") for i in range(2)]
            _gB = [psum.tile([128, 256], dt.float32, tag=f"gB{i}",
                             bufs=1, name=f"gB{i}") for i in range(2)]
            # tile A holds g~(row96), f(row32), i(row0); tile B holds
            # o(row64) alone, so sigma(o) is the only activation gated on
            # the full matmul burst — the rest of the cell update runs
            # while the o-group matmuls stream
            gtile = [_gA, _gA, _gB, _gA]
            # filler target: keeps the PE pstate warm across the
            # AllGather window (never read)
            fpp = psum.tile([128, 512], dt.float32, tag="fpp", bufs=1,
                            name="fpp")
            flh = state.tile([128, 1], dt.bfloat16)

            nc.sync.dma_start(wrec[:], wrec_d[:])
            nc.sync.dma_start(wx[:], wx_d[:])
            nc.sync.dma_start(wout[:], wout_d[:])
            nc.sync.dma_start(xp[:], xp_d[:])
            nc.vector.memset(cst[:], 0.0)
            nc.vector.memset(flh[:], 0.0)

            NFILL = 103
            for s in range(nsteps):
                W = wx if s == 0 else wrec
                # group-major in order g~(3), f(1), i(0), o(2): each
                # gate's nonlinearity and the c update run as soon as
                # that group's matmuls finish, overlapping the rest of
                # the burst; only sigma(o) and the final h mul trail it.
                # bias/ones chunk first within each group (no dependency
                # on the exchanged h, so the PE can issue it while the
                # AllGather is still in flight)
                for gi in [3, 1, 0, 2]:
                    dst = gtile[gi][s % 2]
                    for k in [KC] + list(range(KC)):
                        if s == 0:
                            lhsT = xp[:, k:k + 1]
                        elif k < KC:
                            lhsT = hist[:, s - 1, k:k + 1]
                        else:
                            lhsT = xp[:, KC:KC + 1]
                        nc.tensor.matmul(
                            dst[32 * gi:32 * gi + 1, :], lhsT,
                            W[:, k, 256 * gi:256 * gi + 256],
                            start=(k == KC), stop=(k == KC - 1),
                            tile_position=(0, 32 * gi))

                # ---- LSTM cell elementwise (all operands base 0) ----
                p2 = s % 2
                t_g = work.tile([1, UPC], dt.float32, tag="tg")
                s_f = work.tile([1, UPC], dt.float32, tag="sf")
                s_i = work.tile([1, UPC], dt.float32, tag="si")
                s_o = work.tile([1, UPC], dt.float32, tag="so")
                nc.scalar.activation(t_g[:], gtile[3][p2][96:97, :], Tanh)
                nc.scalar.activation(s_f[:], gtile[1][p2][32:33, :], Sig)
                m_fc = work.tile([1, UPC], dt.float32, tag="mfc")
                nc.vector.tensor_mul(m_fc[:], s_f[:], cst[:])
                nc.scalar.activation(s_i[:], gtile[0][p2][0:1, :], Sig)
                m_ig = work.tile([1, UPC], dt.float32, tag="mig")
                nc.vector.tensor_mul(m_ig[:], s_i[:], t_g[:])
                nc.vector.tensor_add(cst[:], m_ig[:], m_fc[:])
                t_c = work.tile([1, UPC], dt.float32, tag="tc")
                nc.scalar.activation(t_c[:], cst[:], Tanh)
                nc.scalar.activation(s_o[:], gtile[2][p2][64:65, :], Sig)
                h_bf = work.tile([1, UPC], dt.bfloat16, tag="hbf")
                nc.vector.tensor_mul(h_bf[:], s_o[:], t_c[:])

                inb = dram.tile([1, UPC], dt.bfloat16, tag="inb")
                outb = dram.tile([128, KC], dt.bfloat16, tag="outb")
                nc.sync.dma_start(inb[:], h_bf[:])
                nc.gpsimd.collective_compute(
                    "AllGather", mybir.AluOpType.bypass,
                    ins=[inb.opt()], outs=[outb.opt()], replica_groups=rg)
                nc.sync.dma_start(hist[:, s, :], outb[:])
                if s < nsteps - 1:
                    # dummy matmuls bridge the PE through the AllGather
                    # window so the real matmul burst starts at full
                    # p-state (cost model warm-up)
                    for _ in range(NFILL):
                        nc.tensor.matmul(fpp[0:1, :], flh[:, 0:1],
                                         wrec[:, KC, 0:512],
                                         start=True, stop=True)

            # output projection: y = H @ W_out.T (redundant on every core)
            for sh in range(nsteps // 128):
                for nh in range(2):
                    yp = psum.tile([128, 512], dt.float32, tag="yp")
                    for k in range(KC):
                        nc.tensor.matmul(
                            yp[:], hist[:, sh * 128:(sh + 1) * 128, k],
                            wout[:, k, nh * 512:(nh + 1) * 512],
                            start=(k == 0), stop=(k == KC - 1))
                    y_sb = work.tile([128, 512], dt.float32, tag="ysb")
                    nc.vector.tensor_copy(y_sb[:], yp[:])
                    nc.sync.dma_start(
                        y_d[sh * 128:(sh + 1) * 128,
                            nh * 512:(nh + 1) * 512],
                        y_sb[:])

    nc.compile()
    return nc


def kernel(x, W_ih, W_hh, b_ih, b_hh, W_out, b_out, seq_len, _trace=False):
    from concourse.bass_utils import run_bass_kernel_spmd

    assert int(seq_len) == L
    x = np.asarray(x, np.float32)
    W_ih = np.asarray(W_ih, np.float32)
    W_hh = np.asarray(W_hh, np.float32)
    b = np.asarray(b_ih, np.float32) + np.asarray(b_hh, np.float32)
    W_out = np.asarray(W_out, np.float32)
    b_out = np.asarray(b_out, np.float32)

    in_maps = _prep_core_inputs(x, W_ih, W_ih + W_hh, b, W_out)
    nc = _build_program()
    res = run_bass_kernel_spmd(nc, in_maps, list(range(NCORES)),
                               trace=_trace)
    y = np.asarray(res.results[0]["y"], np.float32) + b_out
    out = y[None]  # [1, L, DOUT]
    if _trace:
        return out, res
    return out
